# revision 1
# baseline (speedup 1.0000x reference)
"""Trainium2 Bass kernel for EnhancedCondConv2d (moe_routing).

Data-parallel over batch: 8 cores x 2 samples each. Full inputs in,
full outputs back.

Per-core program (per sample):
  1. routing: avgpool(x) -> tiny MLP -> softmax -> rweights [16]
  2. w[b] = sum_e rweights[e] * experts[e]  (block-diag PE matmuls)
  3. 3x3 grouped conv as 9 PSUM-accumulated shifted matmuls (float32r)
  4. SE: channel mean folded into PSUM eviction (ACT accum), MLP -> cw,
     in-place scale pass out *= cw
  5. CBAM: PE transposes -> DVE max / DVE sum over channels -> 7x7 conv
     as 14 banded-Toeplitz matmuls -> sigmoid -> sw
  6. final: out * sw_broadcast + x, DMA out
"""

import math
from contextlib import ExitStack

import numpy as np

import concourse.bass as bass
import concourse.bacc as bacc
import concourse.mybir as mybir
import concourse.tile as tile
from concourse.bass_utils import run_bass_kernel_spmd

F32 = mybir.dt.float32
F32R = mybir.dt.float32r
BF16 = mybir.dt.bfloat16
AX = mybir.AxisListType
ALU = mybir.AluOpType
ACTF = mybir.ActivationFunctionType

B, CI, CO, H, W, E, KK, RR = 16, 128, 128, 128, 128, 16, 3, 8
NCORES = 8
BL = B // NCORES  # 2 samples per core
EPS = 1e-5
HW = H * W
IKK = CI * KK * KK  # 1152
BNS = 1.0 / math.sqrt(1.0 + EPS)

_CACHE = {}


def _build_module():
    nc = bacc.Bacc("TRN2", target_bir_lowering=False, debug=False)

    # ---- external inputs (host-prepped layouts) ----
    x_d = nc.dram_tensor("x2", [BL, CI, H, W], F32, kind="ExternalInput").ap()
    xb_d = nc.dram_tensor("x2b", [BL, CI, H, W], BF16, kind="ExternalInput").ap()
    ew_d = nc.dram_tensor("experts_w", [16, 128, IKK], BF16, kind="ExternalInput").ap()
    wid_d = nc.dram_tensor("wident", [128, 134], F32, kind="ExternalInput").ap()
    rw1t_d = nc.dram_tensor("rw1t", [CI, 16], F32, kind="ExternalInput").ap()
    rw2t_d = nc.dram_tensor("rw2t", [16, CI], F32, kind="ExternalInput").ap()
    rw3t_d = nc.dram_tensor("rw3t", [CI, 16], F32, kind="ExternalInput").ap()
    caw1t_d = nc.dram_tensor("caw1t", [CO, 16], F32, kind="ExternalInput").ap()
    caw2t_d = nc.dram_tensor("caw2t", [16, CO], F32, kind="ExternalInput").ap()
    g1_d = nc.dram_tensor("rbn1_g", [16], F32, kind="ExternalInput").ap()
    b1_d = nc.dram_tensor("rbn1_b", [16], F32, kind="ExternalInput").ap()
    g2_d = nc.dram_tensor("rbn2_g", [CI], F32, kind="ExternalInput").ap()
    b2_d = nc.dram_tensor("rbn2_b", [CI], F32, kind="ExternalInput").ap()
    rb3_d = nc.dram_tensor("rb3", [E], F32, kind="ExternalInput").ap()
    cag1_d = nc.dram_tensor("ca_bn1_g", [16], F32, kind="ExternalInput").ap()
    cab1_d = nc.dram_tensor("ca_bn1_b", [16], F32, kind="ExternalInput").ap()
    cag2_d = nc.dram_tensor("ca_bn2_g", [CO], F32, kind="ExternalInput").ap()
    cab2_d = nc.dram_tensor("ca_bn2_b", [CO], F32, kind="ExternalInput").ap()
    saw_d = nc.dram_tensor("sawf", [98], F32, kind="ExternalInput").ap()
    sag_d = nc.dram_tensor("sa_bn_g", [1], F32, kind="ExternalInput").ap()
    sab_d = nc.dram_tensor("sa_bn_b", [1], F32, kind="ExternalInput").ap()
    bmask_d = nc.dram_tensor("bmask", [128, 8], BF16, kind="ExternalInput").ap()

    out_d = nc.dram_tensor("out", [BL, CO, H, W], F32, kind="ExternalOutput").ap()

    # internal DRAM scratch
    srw_d = nc.dram_tensor("scr_rw", [BL, E], F32).ap()
    ssw_d = nc.dram_tensor("scr_sw", [BL, H, W], BF16).ap()

    with tile.TileContext(nc) as tc, ExitStack() as ctx:
        _kernel_body(
            ctx, tc,
            x_d, xb_d, ew_d, wid_d, rw1t_d, rw2t_d, rw3t_d, caw1t_d, caw2t_d,
            g1_d, b1_d, g2_d, b2_d, rb3_d, cag1_d, cab1_d, cag2_d, cab2_d,
            saw_d, sag_d, sab_d, bmask_d, out_d, srw_d, ssw_d,
        )
    nc.compile()
    return nc


def _kernel_body(ctx, tc,
                 x_d, xb_d, ew_d, wid_d, rw1t_d, rw2t_d, rw3t_d, caw1t_d, caw2t_d,
                 g1_d, b1_d, g2_d, b2_d, rb3_d, cag1_d, cab1_d, cag2_d, cab2_d,
                 saw_d, sag_d, sab_d, bmask_d, out_d, srw_d, ssw_d):
    nc = tc.nc

    cpool = ctx.enter_context(tc.tile_pool(name="const", bufs=1))
    xpool = ctx.enter_context(tc.tile_pool(name="xp", bufs=1))
    opool = ctx.enter_context(tc.tile_pool(name="op", bufs=1))
    wpool = ctx.enter_context(tc.tile_pool(name="wp", bufs=1))
    epool = ctx.enter_context(tc.tile_pool(name="ep", bufs=3))
    spool = ctx.enter_context(tc.tile_pool(name="sp", bufs=1))
    fpool = ctx.enter_context(tc.tile_pool(name="fp", bufs=3))
    scpool = ctx.enter_context(tc.tile_pool(name="scr", bufs=2))

    pconv = ctx.enter_context(tc.tile_pool(name="pc", bufs=4, space="PSUM"))
    pw = ctx.enter_context(tc.tile_pool(name="pw", bufs=3, space="PSUM"))
    pr = ctx.enter_context(tc.tile_pool(name="prt", bufs=1, space="PSUM"))

    # ---------- constants ----------
    wident = cpool.tile([128, 134], F32, tag="wident")
    nc.sync.dma_start(wident, wid_d)
    ident = wident[:, 3:131]

    rw1t = cpool.tile([CI, 16], F32, tag="rw1t")
    nc.sync.dma_start(rw1t, rw1t_d)
    rw2t = cpool.tile([16, CI], F32, tag="rw2t")
    nc.sync.dma_start(rw2t, rw2t_d)
    rw3t = cpool.tile([CI, 16], F32, tag="rw3t")
    nc.sync.dma_start(rw3t, rw3t_d)
    caw1t = cpool.tile([CO, 16], F32, tag="caw1t")
    nc.sync.dma_start(caw1t, caw1t_d)
    caw2t = cpool.tile([16, CO], F32, tag="caw2t")
    nc.sync.dma_start(caw2t, caw2t_d)

    def vec_const(dst_tag, src_ap, n, scale):
        raw = cpool.tile([n, 1], F32, tag=dst_tag + "_r")
        nc.sync.dma_start(raw, src_ap.unsqueeze(1))
        out = cpool.tile([n, 1], F32, tag=dst_tag)
        nc.vector.tensor_scalar_mul(out, raw, float(scale))
        return out

    gs1 = vec_const("gs1", g1_d, 16, BNS / HW)
    bb1 = vec_const("bb1", b1_d, 16, 1.0)
    gs2 = vec_const("gs2", g2_d, CI, BNS)
    bb2 = vec_const("bb2", b2_d, CI, 1.0)
    gsca1 = vec_const("gsca1", cag1_d, 16, BNS / HW)
    bbca1 = vec_const("bbca1", cab1_d, 16, 1.0)
    gsca2 = vec_const("gsca2", cag2_d, CO, BNS)
    bbca2 = vec_const("bbca2", cab2_d, CO, 1.0)

    rb3r = cpool.tile([1, E], F32, tag="rb3r")
    nc.sync.dma_start(rb3r, rb3_d.unsqueeze(0))

    # spatial-attention 7x7 taps, broadcast to all partitions
    sabc = cpool.tile([128, 98], F32, tag="sabc")
    nc.sync.dma_start(sabc, saw_d.unsqueeze(0).partition_broadcast(128))
    sak = cpool.tile([128, 98], F32, tag="sak")
    # mean channel (c=0) carries the 1/CO normalization of the channel-mean
    nc.vector.tensor_scalar_mul(sak[:, 0:49], sabc[:, 0:49], 1.0 / CO)
    nc.vector.tensor_copy(sak[:, 49:98], sabc[:, 49:98])

    gssa = cpool.tile([128, 1], F32, tag="gssa")
    nc.sync.dma_start(gssa, sag_d.unsqueeze(0).partition_broadcast(128))
    nc.vector.tensor_scalar_mul(gssa, gssa, BNS)
    bssa = cpool.tile([128, 1], F32, tag="bssa")
    nc.sync.dma_start(bssa, sab_d.unsqueeze(0).partition_broadcast(128))
    bmask = cpool.tile([128, 8], BF16, tag="bmask")
    nc.sync.dma_start(bmask, bmask_d)

    # banded Toeplitz matrices M[c,dh][k, w] = sum_dw sak[c,dh,dw] * S_dw[k, w]
    mcdh = []
    msA = cpool.tile([128, 128], F32, tag="msA")
    msB = cpool.tile([128, 128], F32, tag="msB")
    for t in range(14):
        c, dh = t // 7, t % 7
        dst = cpool.tile([128, 128], F32, tag=f"mcdh{t}")
        mcdh.append(dst)
        chain = [msA, msB, msA, msB, msA, msB, dst]
        for dw in range(7):
            sidx = c * 49 + dh * 7 + dw
            sc = sak[:, sidx:sidx + 1]
            shift = wident[:, dw:dw + 128]
            if dw == 0:
                nc.vector.tensor_scalar_mul(chain[0], shift, sc)
            else:
                nc.vector.scalar_tensor_tensor(
                    chain[dw], shift, sc, chain[dw - 1], ALU.mult, ALU.add)

    # ---------- per-sample ----------
    for b in range(BL):
        # -- load x (padded) --
        xp = xpool.tile([128, H + 2, W + 2], BF16, tag="x_pad")
        nc.vector.memset(xp[:, 0, :], 0.0)
        nc.vector.memset(xp[:, H + 1, :], 0.0)
        nc.vector.memset(xp[:, 1:H + 1, 0], 0.0)
        nc.vector.memset(xp[:, 1:H + 1, W + 1], 0.0)
        nc.sync.dma_start(xp[:, 1:H + 1, 1:W + 1], xb_d[b])

        # -- avgpool (sum; mean folded into BN scale) --
        psum_a = spool.tile([128, 1], F32, tag="psum_a")
        nc.vector.tensor_reduce(psum_a, xp[:, 1:65, 1:W + 1], AX.XY, ALU.add)
        pparts = spool.tile([128, 16], F32, tag="pparts")
        for i in range(16):
            pscr = scpool.tile([128, 4, 128], F32, tag="pscr")
            nc.scalar.activation(
                pscr, xp[:, 65 + 4 * i:69 + 4 * i, 1:W + 1], ACTF.Copy,
                accum_out=pparts[:, i:i + 1])
        psum_b = spool.tile([128, 1], F32, tag="psum_b")
        nc.vector.tensor_reduce(psum_b, pparts, AX.X, ALU.add)
        psum_t = spool.tile([128, 1], F32, tag="psum_t")
        nc.vector.tensor_add(psum_t, psum_a, psum_b)

        # -- routing MLP --
        mm1 = pr.tile([16, 1], F32, tag="r")
        nc.tensor.matmul(mm1, rw1t, psum_t, start=True, stop=True)
        h1 = spool.tile([16, 1], F32, tag="h1")
        nc.scalar.activation(h1, mm1, ACTF.Relu, bias=bb1, scale=gs1)
        mm2 = pr.tile([128, 1], F32, tag="r")
        nc.tensor.matmul(mm2, rw2t, h1, start=True, stop=True)
        gg = spool.tile([128, 1], F32, tag="gg")
        nc.scalar.activation(gg, mm2, ACTF.Sigmoid, bias=bb2, scale=gs2)
        mm3 = pr.tile([1, E], F32, tag="r")
        nc.tensor.matmul(mm3, gg, rw3t, start=True, stop=True)
        lg = spool.tile([1, E], F32, tag="lg")
        nc.vector.tensor_add(lg, mm3, rb3r)
        mx = spool.tile([1, 1], F32, tag="mx")
        nc.vector.tensor_reduce(mx, lg, AX.X, ALU.max)
        mxn = spool.tile([1, 1], F32, tag="mxn")
        nc.vector.tensor_scalar_mul(mxn, mx, -1.0)
        e16 = spool.tile([1, E], F32, tag="e16")
        nc.scalar.activation(e16, lg, ACTF.Exp, bias=mxn, scale=1.0)
        s1 = spool.tile([1, 1], F32, tag="s1")
        nc.vector.tensor_reduce(s1, e16, AX.X, ALU.add)
        rinv = spool.tile([1, 1], F32, tag="rinv")
        nc.vector.reciprocal(rinv, s1)
        rwrow = spool.tile([1, E], F32, tag="rwrow")
        nc.vector.tensor_scalar_mul(rwrow, e16, rinv)
        nc.sync.dma_start(srw_d[b].unsqueeze(0), rwrow)

        # block-diag routing weights [ (j,e)=128, j'=8 ]
        rwcol = spool.tile([128, 1], F32, tag="rwcol")
        nc.sync.dma_start(
            rwcol, srw_d[b].unsqueeze(0).broadcast_to([8, E]))
        rwblk = spool.tile([128, 8], BF16, tag="rwblk")
        nc.vector.tensor_scalar_mul(rwblk, bmask, rwcol)

        # -- w generation: w[i, k, o] = sum_e rw[e] experts[e, o, i, k] --
        wsb = wpool.tile([128, KK * KK, CO], BF16, tag="wsb")
        pwt = [pw.tile([128, 384], F32, tag="w", name=f"pw{b}_{i}") for i in range(3)]
        for og in range(16):
            ec = epool.tile([128, IKK], BF16, tag="echunk")
            nc.sync.dma_start(ec, ew_d[og])
            eck = ec.rearrange("p (i k) -> p k i", k=9)
            for k in range(9):
                lhs = eck[:, k, :]
                dst = pwt[k // 3][:, (k % 3) * 128 + og * 8:(k % 3) * 128 + og * 8 + 8]
                nc.tensor.matmul(dst, lhs, rwblk,
                                 start=True, stop=True)
        for k in range(9):
            nc.vector.tensor_copy(
                wsb[:, k, :], pwt[k // 3][:, (k % 3) * 128:(k % 3) * 128 + 128])

        # -- conv: 8 supers x 4 groups x 9 taps --
        osb = opool.tile([128, H, W], F32, tag="out_sb")
        cparts = spool.tile([128, 32], F32, tag="cparts")
        for sup in range(8):
            pcs = [pconv.tile([128, 512], F32, tag="c", name=f"pc{b}_{sup}_{i}")
                   for i in range(4)]
            for k in range(9):
                kh, kw = k // 3, k % 3
                lhs = wsb[:, k, :]
                for g in range(4):
                    r0 = sup * 16 + g * 4 + kh
                    rhs = xp[:, r0:r0 + 4, kw:kw + W]
                    nc.tensor.matmul(pcs[g], lhs, rhs,
                                     start=(k == 0), stop=(k == 8))
            for g in range(4):
                hr = sup * 16 + g * 4
                nc.scalar.activation(
                    osb[:, hr:hr + 4, :], pcs[g].rearrange("p (a b) -> p a b", a=4),
                    ACTF.Copy, accum_out=cparts[:, sup * 4 + g:sup * 4 + g + 1])

        # -- SE --
        cps = spool.tile([128, 1], F32, tag="cps")
        nc.vector.tensor_reduce(cps, cparts, AX.X, ALU.add)
        se1 = pr.tile([16, 1], F32, tag="r")
        nc.tensor.matmul(se1, caw1t, cps, start=True, stop=True)
        ch = spool.tile([16, 1], F32, tag="ch")
        nc.scalar.activation(ch, se1, ACTF.Relu, bias=bbca1, scale=gsca1)
        se2 = pr.tile([128, 1], F32, tag="r")
        nc.tensor.matmul(se2, caw2t, ch, start=True, stop=True)
        cw = spool.tile([128, 1], F32, tag="cw")
        nc.scalar.activation(cw, se2, ACTF.Sigmoid, bias=bbca2, scale=gsca2)

        # in-place SE scale of conv output
        for g in range(32):
            nc.scalar.mul(osb[:, 4 * g:4 * g + 4, :], osb[:, 4 * g:4 * g + 4, :], cw)

        # -- CBAM stats: transpose chunks, reduce over channels --
        spmax = spool.tile([128, 134], F32, tag="spmax")
        spsum = spool.tile([128, 134], F32, tag="spsum")
        nc.vector.memset(spmax[:, 0:3], 0.0)
        nc.vector.memset(spmax[:, 131:134], 0.0)
        nc.vector.memset(spsum[:, 0:3], 0.0)
        nc.vector.memset(spsum[:, 131:134], 0.0)
        for q in range(32):
            ptt = pconv.tile([128, 512], F32, tag="c")
            for j in range(4):
                chh = 4 * q + j
                nc.tensor.transpose(
                    ptt[:, 128 * j:128 * (j + 1)], osb[:, chh, :], ident)
            v = ptt.rearrange("p (a b) -> p a b", a=4)
            nc.vector.tensor_reduce(spmax[:, 3 + 4 * q:7 + 4 * q], v, AX.X, ALU.max)
            nc.vector.tensor_reduce(spsum[:, 3 + 4 * q:7 + 4 * q], v, AX.X, ALU.add)

        # -- CBAM 7x7 conv: 14 banded matmuls --
        psw = pconv.tile([128, 128], F32, tag="c")
        for t in range(14):
            c, dh = t // 7, t % 7
            src = spsum if c == 0 else spmax
            nc.tensor.matmul(psw, mcdh[t], src[:, dh:dh + 128],
                             start=(t == 0), stop=(t == 13))
        swT = spool.tile([128, 128], F32, tag="swT")
        nc.scalar.activation(swT, psw, ACTF.Sigmoid, bias=bssa, scale=gssa)
        pswh = pconv.tile([128, 128], F32, tag="c")
        nc.tensor.transpose(pswh, swT, ident)
        swH = spool.tile([128, 128], BF16, tag="swH")
        nc.vector.tensor_copy(swH, pswh)
        nc.sync.dma_start(ssw_d[b], swH)

        # -- final: out = out*sw + x --
        for g in range(32):
            swbc = fpool.tile([128, 4, 128], BF16, tag="swbc")
            nc.sync.dma_start(
                swbc, ssw_d[b, 4 * g:4 * g + 4, :].partition_broadcast(128))
            tmul = fpool.tile([128, 4, 128], F32, tag="tmul")
            nc.vector.tensor_mul(tmul, osb[:, 4 * g:4 * g + 4, :], swbc)
            xres = fpool.tile([128, 4, 128], F32, tag="xres")
            nc.sync.dma_start(xres, x_d[b, :, 4 * g:4 * g + 4, :])
            fo = fpool.tile([128, 4, 128], F32, tag="fo")
            eng = nc.vector if (g % 2 == 0) else nc.gpsimd
            eng.tensor_tensor(fo, tmul, xres, ALU.add)
            nc.sync.dma_start(out_d[b, :, 4 * g:4 * g + 4, :], fo)


def _host_prep(inp):
    import ml_dtypes
    experts = np.ascontiguousarray(inp["experts"], dtype=np.float32)
    ew = experts.reshape(E, CO, IKK).reshape(E, 16, 8, IKK)
    ew = np.ascontiguousarray(ew.transpose(1, 2, 0, 3)).reshape(16, 128, IKK)

    wid = np.zeros((128, 134), dtype=np.float32)
    wid[np.arange(128), np.arange(128) + 3] = 1.0

    sawf = np.ascontiguousarray(inp["sa_w"].reshape(2, 49)).reshape(98)

    bm = np.zeros((8, 16, 8), dtype=ml_dtypes.bfloat16)
    for j in range(8):
        bm[j, :, j] = 1.0
    bm = bm.reshape(128, 8)

    import ml_dtypes
    shared = {
        "experts_w": ew.astype(ml_dtypes.bfloat16),
        "wident": wid,
        "rw1t": np.ascontiguousarray(inp["rw1"].T, dtype=np.float32),
        "rw2t": np.ascontiguousarray(inp["rw2"].T, dtype=np.float32),
        "rw3t": np.ascontiguousarray(inp["rw3"].T, dtype=np.float32),
        "caw1t": np.ascontiguousarray(inp["ca_w1"].T, dtype=np.float32),
        "caw2t": np.ascontiguousarray(inp["ca_w2"].T, dtype=np.float32),
        "rbn1_g": np.asarray(inp["rbn1_g"], np.float32),
        "rbn1_b": np.asarray(inp["rbn1_b"], np.float32),
        "rbn2_g": np.asarray(inp["rbn2_g"], np.float32),
        "rbn2_b": np.asarray(inp["rbn2_b"], np.float32),
        "rb3": np.asarray(inp["rb3"], np.float32),
        "ca_bn1_g": np.asarray(inp["ca_bn1_g"], np.float32),
        "ca_bn1_b": np.asarray(inp["ca_bn1_b"], np.float32),
        "ca_bn2_g": np.asarray(inp["ca_bn2_g"], np.float32),
        "ca_bn2_b": np.asarray(inp["ca_bn2_b"], np.float32),
        "sawf": np.asarray(sawf, np.float32),
        "sa_bn_g": np.asarray(inp["sa_bn_g"], np.float32),
        "sa_bn_b": np.asarray(inp["sa_bn_b"], np.float32),
        "bmask": bm,
    }
    x = np.asarray(inp["x"], np.float32)
    in_maps = []
    for c in range(NCORES):
        m = dict(shared)
        xc = np.ascontiguousarray(x[BL * c:BL * (c + 1)])
        m["x2"] = xc
        m["x2b"] = xc.astype(ml_dtypes.bfloat16)
        in_maps.append(m)
    return in_maps


def get_module():
    if "nc" not in _CACHE:
        _CACHE["nc"] = _build_module()
    return _CACHE["nc"]


def kernel(**inputs):
    nc = get_module()
    in_maps = _host_prep(inputs)
    res = run_bass_kernel_spmd(nc, in_maps, core_ids=list(range(NCORES)))
    out = np.concatenate([r["out"] for r in res.results], axis=0)
    return out.astype(np.float32)



# revision 4
# speedup vs baseline: 1.4665x; 1.4665x over previous
"""Trainium2 Bass kernel for EnhancedCondConv2d (moe_routing).

Data-parallel over batch: 8 cores x 2 samples each. Full inputs in,
full outputs back.

Per-core program (per sample):
  1. routing: avgpool(x) -> tiny MLP -> softmax -> rweights [16]
  2. w[b] = sum_e rweights[e] * experts[e]  (block-diag PE matmuls)
  3. 3x3 grouped conv as 9 PSUM-accumulated shifted matmuls (bf16)
  4. SE: channel mean folded into PSUM eviction (ACT accum), MLP -> cw,
     in-place bf16 DVE scale pass osb *= cw
  5. CBAM: PE transposes (bf16) -> DVE max / sum over channels -> 7x7
     conv as 14 banded-Toeplitz matmuls (host-precomputed bands)
  6. final: out * sw_broadcast + x (residual read from the bf16 padded
     x already in SBUF), DMA out

The two samples are software-pipelined: sample 1's weight generation and
conv overlap sample 0's SE/CBAM/final phases.
"""

import math
from contextlib import ExitStack

import numpy as np

import concourse.bass as bass
import concourse.bacc as bacc
import concourse.mybir as mybir
import concourse.tile as tile
from concourse.bass_utils import run_bass_kernel_spmd

F32 = mybir.dt.float32
BF16 = mybir.dt.bfloat16
AX = mybir.AxisListType
ALU = mybir.AluOpType
ACTF = mybir.ActivationFunctionType

B, CI, CO, H, W, E, KK, RR = 16, 128, 128, 128, 128, 16, 3, 8
NCORES = 8
BL = B // NCORES  # 2 samples per core
EPS = 1e-5
HW = H * W
IKK = CI * KK * KK  # 1152
BNS = 1.0 / math.sqrt(1.0 + EPS)

_CACHE = {}


def _build_module():
    nc = bacc.Bacc("TRN2", target_bir_lowering=False, debug=False)

    # ---- external inputs (host-prepped layouts) ----
    xb_d = nc.dram_tensor("x2b", [BL, CI, H, W], BF16, kind="ExternalInput").ap()
    ew_d = nc.dram_tensor("experts_w", [16, 128, IKK], BF16, kind="ExternalInput").ap()
    ident_d = nc.dram_tensor("identb", [128, 128], BF16, kind="ExternalInput").ap()
    mcdh_d = nc.dram_tensor("mcdh", [128, 14 * 128], BF16, kind="ExternalInput").ap()
    rw1t_d = nc.dram_tensor("rw1t", [CI, 16], F32, kind="ExternalInput").ap()
    rw2t_d = nc.dram_tensor("rw2t", [16, CI], F32, kind="ExternalInput").ap()
    rw3t_d = nc.dram_tensor("rw3t", [CI, 16], F32, kind="ExternalInput").ap()
    caw1t_d = nc.dram_tensor("caw1t", [CO, 16], F32, kind="ExternalInput").ap()
    caw2t_d = nc.dram_tensor("caw2t", [16, CO], F32, kind="ExternalInput").ap()
    g1_d = nc.dram_tensor("rbn1_g", [16], F32, kind="ExternalInput").ap()
    b1_d = nc.dram_tensor("rbn1_b", [16], F32, kind="ExternalInput").ap()
    g2_d = nc.dram_tensor("rbn2_g", [CI], F32, kind="ExternalInput").ap()
    b2_d = nc.dram_tensor("rbn2_b", [CI], F32, kind="ExternalInput").ap()
    rb3_d = nc.dram_tensor("rb3", [E], F32, kind="ExternalInput").ap()
    cag1_d = nc.dram_tensor("ca_bn1_g", [16], F32, kind="ExternalInput").ap()
    cab1_d = nc.dram_tensor("ca_bn1_b", [16], F32, kind="ExternalInput").ap()
    cag2_d = nc.dram_tensor("ca_bn2_g", [CO], F32, kind="ExternalInput").ap()
    cab2_d = nc.dram_tensor("ca_bn2_b", [CO], F32, kind="ExternalInput").ap()
    sag_d = nc.dram_tensor("sa_bn_g", [1], F32, kind="ExternalInput").ap()
    sab_d = nc.dram_tensor("sa_bn_b", [1], F32, kind="ExternalInput").ap()
    bmask_d = nc.dram_tensor("bmask", [128, 8], BF16, kind="ExternalInput").ap()

    out_d = nc.dram_tensor("out", [BL, CO, H, W], F32, kind="ExternalOutput").ap()

    # internal DRAM scratch
    srw_d = nc.dram_tensor("scr_rw", [BL, E], F32).ap()
    ssw_d = nc.dram_tensor("scr_sw", [BL, H, W], BF16).ap()

    with tile.TileContext(nc) as tc, ExitStack() as ctx:
        _kernel_body(
            ctx, tc,
            xb_d, ew_d, ident_d, mcdh_d, rw1t_d, rw2t_d, rw3t_d, caw1t_d,
            caw2t_d, g1_d, b1_d, g2_d, b2_d, rb3_d, cag1_d, cab1_d, cag2_d,
            cab2_d, sag_d, sab_d, bmask_d, out_d, srw_d, ssw_d,
        )
    nc.compile()
    return nc


def _kernel_body(ctx, tc,
                 xb_d, ew_d, ident_d, mcdh_d, rw1t_d, rw2t_d, rw3t_d, caw1t_d,
                 caw2t_d, g1_d, b1_d, g2_d, b2_d, rb3_d, cag1_d, cab1_d,
                 cag2_d, cab2_d, sag_d, sab_d, bmask_d, out_d, srw_d, ssw_d):
    nc = tc.nc

    cpool = ctx.enter_context(tc.tile_pool(name="const", bufs=1))
    xpool = ctx.enter_context(tc.tile_pool(name="xp", bufs=2))
    opool = ctx.enter_context(tc.tile_pool(name="op", bufs=2))
    wpool = ctx.enter_context(tc.tile_pool(name="wp", bufs=2))
    epool = ctx.enter_context(tc.tile_pool(name="ep", bufs=3))
    spool = ctx.enter_context(tc.tile_pool(name="sp", bufs=2))
    fpool = ctx.enter_context(tc.tile_pool(name="fp", bufs=2))

    pconv = ctx.enter_context(tc.tile_pool(name="pc", bufs=5, space="PSUM"))
    pw = ctx.enter_context(tc.tile_pool(name="pw", bufs=2, space="PSUM"))
    pr = ctx.enter_context(tc.tile_pool(name="prt", bufs=1, space="PSUM"))

    # ---------- constants ----------
    ident = cpool.tile([128, 128], BF16, tag="ident")
    nc.sync.dma_start(ident, ident_d)
    mcdh = cpool.tile([128, 14 * 128], BF16, tag="mcdh")
    nc.sync.dma_start(mcdh, mcdh_d)

    rw1t = cpool.tile([CI, 16], F32, tag="rw1t")
    nc.sync.dma_start(rw1t, rw1t_d)
    rw2t = cpool.tile([16, CI], F32, tag="rw2t")
    nc.sync.dma_start(rw2t, rw2t_d)
    rw3t = cpool.tile([CI, 16], F32, tag="rw3t")
    nc.sync.dma_start(rw3t, rw3t_d)
    caw1t = cpool.tile([CO, 16], F32, tag="caw1t")
    nc.sync.dma_start(caw1t, caw1t_d)
    caw2t = cpool.tile([16, CO], F32, tag="caw2t")
    nc.sync.dma_start(caw2t, caw2t_d)

    def vec_const(dst_tag, src_ap, n, scale):
        raw = cpool.tile([n, 1], F32, tag=dst_tag + "_r")
        nc.sync.dma_start(raw, src_ap.unsqueeze(1))
        out = cpool.tile([n, 1], F32, tag=dst_tag)
        nc.vector.tensor_scalar_mul(out, raw, float(scale))
        return out

    gs1 = vec_const("gs1", g1_d, 16, BNS / HW)
    bb1 = vec_const("bb1", b1_d, 16, 1.0)
    gs2 = vec_const("gs2", g2_d, CI, BNS)
    bb2 = vec_const("bb2", b2_d, CI, 1.0)
    gsca1 = vec_const("gsca1", cag1_d, 16, BNS / HW)
    bbca1 = vec_const("bbca1", cab1_d, 16, 1.0)
    gsca2 = vec_const("gsca2", cag2_d, CO, BNS)
    bbca2 = vec_const("bbca2", cab2_d, CO, 1.0)

    rb3r = cpool.tile([1, E], F32, tag="rb3r")
    nc.sync.dma_start(rb3r, rb3_d.unsqueeze(0))

    gssa = cpool.tile([128, 1], F32, tag="gssa")
    nc.sync.dma_start(gssa, sag_d.unsqueeze(0).partition_broadcast(128))
    nc.vector.tensor_scalar_mul(gssa, gssa, BNS)
    bssa = cpool.tile([128, 1], F32, tag="bssa")
    nc.sync.dma_start(bssa, sab_d.unsqueeze(0).partition_broadcast(128))
    bmask = cpool.tile([128, 8], BF16, tag="bmask")
    nc.sync.dma_start(bmask, bmask_d)

    # ---------- per-sample stage helpers ----------
    xps = [None, None]
    osbs = [None, None]
    wsbs = [None, None]
    cparts = [None, None]

    def stage_load_pool_route(b):
        """x load (chunked, pipelined with pooling), routing -> rwblk."""
        xp = xpool.tile([128, H + 2, W + 2], BF16, tag="x_pad")
        xps[b] = xp
        nc.vector.memset(xp[:, 0, :], 0.0)
        nc.vector.memset(xp[:, H + 1, :], 0.0)
        nc.vector.memset(xp[:, 1:H + 1, 0], 0.0)
        nc.vector.memset(xp[:, 1:H + 1, W + 1], 0.0)
        pparts = spool.tile([128, 4], F32, tag="pparts")
        for t in range(4):
            r0 = 32 * t
            nc.sync.dma_start(xp[:, 1 + r0:1 + r0 + 32, 1:W + 1],
                              xb_d[b, :, r0:r0 + 32, :])
            nc.vector.tensor_reduce(pparts[:, t:t + 1],
                                    xp[:, 1 + r0:1 + r0 + 32, 1:W + 1],
                                    AX.XY, ALU.add)
        psum_t = spool.tile([128, 1], F32, tag="psum_t")
        nc.vector.tensor_reduce(psum_t, pparts, AX.X, ALU.add)

        # -- routing MLP (f32) --
        mm1 = pr.tile([16, 1], F32, tag="r")
        nc.tensor.matmul(mm1, rw1t, psum_t, start=True, stop=True)
        h1 = spool.tile([16, 1], F32, tag="h1")
        nc.scalar.activation(h1, mm1, ACTF.Relu, bias=bb1, scale=gs1)
        mm2 = pr.tile([128, 1], F32, tag="r")
        nc.tensor.matmul(mm2, rw2t, h1, start=True, stop=True)
        gg = spool.tile([128, 1], F32, tag="gg")
        nc.scalar.activation(gg, mm2, ACTF.Sigmoid, bias=bb2, scale=gs2)
        mm3 = pr.tile([1, E], F32, tag="r")
        nc.tensor.matmul(mm3, gg, rw3t, start=True, stop=True)
        lg = spool.tile([1, E], F32, tag="lg")
        nc.vector.tensor_add(lg, mm3, rb3r)
        mx = spool.tile([1, 1], F32, tag="mx")
        nc.vector.tensor_reduce(mx, lg, AX.X, ALU.max)
        mxn = spool.tile([1, 1], F32, tag="mxn")
        nc.vector.tensor_scalar_mul(mxn, mx, -1.0)
        e16 = spool.tile([1, E], F32, tag="e16")
        nc.scalar.activation(e16, lg, ACTF.Exp, bias=mxn, scale=1.0)
        s1 = spool.tile([1, 1], F32, tag="s1")
        nc.vector.tensor_reduce(s1, e16, AX.X, ALU.add)
        rinv = spool.tile([1, 1], F32, tag="rinv")
        nc.vector.reciprocal(rinv, s1)
        rwrow = spool.tile([1, E], F32, tag="rwrow")
        nc.vector.tensor_scalar_mul(rwrow, e16, rinv)
        nc.sync.dma_start(srw_d[b].unsqueeze(0), rwrow)

        # block-diag routing weights [ (j,e)=128, j'=8 ]
        rwcol = spool.tile([128, 1], F32, tag="rwcol")
        nc.sync.dma_start(
            rwcol, srw_d[b].unsqueeze(0).broadcast_to([8, E]))
        rwblk = spool.tile([128, 8], BF16, tag="rwblk")
        nc.vector.tensor_scalar_mul(rwblk, bmask, rwcol)
        return rwblk

    def stage_wgen(b, rwblk):
        """w[i, k, o] = sum_e rw[e] experts[e, o, i, k] via block-diag MMs."""
        wsb = wpool.tile([128, KK * KK, CO], BF16, tag="wsb")
        wsbs[b] = wsb
        for og in range(16):
            ec = epool.tile([128, IKK], BF16, tag="echunk")
            nc.sync.dma_start(ec, ew_d[og])
            eck = ec.rearrange("p (k i) -> p k i", k=9)
            pwt = pw.tile([128, 9, 8], F32, tag="w")
            for k in range(9):
                nc.tensor.matmul(pwt[:, k, :], eck[:, k, :], rwblk,
                                 start=True, stop=True)
            nc.scalar.copy(wsb[:, :, og * 8:og * 8 + 8], pwt)

    def stage_conv(b):
        """3x3 conv: 8 supers x 4 groups x 9 taps, evict bf16 + accum."""
        xp, wsb = xps[b], wsbs[b]
        osb = opool.tile([128, H, W], BF16, tag="out_sb")
        osbs[b] = osb
        cp = spool.tile([128, 32], F32, tag="cparts")
        cparts[b] = cp
        for sup in range(8):
            pcs = [pconv.tile([128, 512], F32, tag="c", name=f"pc{b}_{sup}_{i}")
                   for i in range(4)]
            for k in range(9):
                kh, kw = k // 3, k % 3
                lhs = wsb[:, k, :]
                for g in range(4):
                    r0 = sup * 16 + g * 4 + kh
                    rhs = xp[:, r0:r0 + 4, kw:kw + W]
                    nc.tensor.matmul(pcs[g], lhs, rhs,
                                     start=(k == 0), stop=(k == 8))
            for g in range(4):
                hr = sup * 16 + g * 4
                nc.scalar.activation(
                    osb[:, hr:hr + 4, :], pcs[g].rearrange("p (a b) -> p a b", a=4),
                    ACTF.Copy, accum_out=cp[:, sup * 4 + g:sup * 4 + g + 1])

    def stage_se(b):
        """SE channel attention: MLP on accumulated channel sums -> cw,
        then in-place bf16 scale of osb on DVE."""
        osb, cp = osbs[b], cparts[b]
        cps = spool.tile([128, 1], F32, tag="cps")
        nc.vector.tensor_reduce(cps, cp, AX.X, ALU.add)
        se1 = pr.tile([16, 1], F32, tag="r")
        nc.tensor.matmul(se1, caw1t, cps, start=True, stop=True)
        ch = spool.tile([16, 1], F32, tag="ch")
        nc.scalar.activation(ch, se1, ACTF.Relu, bias=bbca1, scale=gsca1)
        se2 = pr.tile([128, 1], F32, tag="r")
        nc.tensor.matmul(se2, caw2t, ch, start=True, stop=True)
        cw = spool.tile([128, 1], F32, tag="cw")
        nc.scalar.activation(cw, se2, ACTF.Sigmoid, bias=bbca2, scale=gsca2)
        for g in range(8):
            nc.vector.tensor_scalar_mul(
                osb[:, 16 * g:16 * g + 16, :], osb[:, 16 * g:16 * g + 16, :], cw)

    def stage_cbam(b):
        """CBAM spatial attention: transpose chunks -> channel max/sum ->
        7x7 conv as 14 banded-Toeplitz matmuls -> sigmoid -> ssw_d."""
        osb = osbs[b]
        spmax = spool.tile([128, 134], BF16, tag="spmax")
        spsum = spool.tile([128, 134], BF16, tag="spsum")
        nc.vector.memset(spmax[:, 0:3], 0.0)
        nc.vector.memset(spmax[:, 131:134], 0.0)
        nc.vector.memset(spsum[:, 0:3], 0.0)
        nc.vector.memset(spsum[:, 131:134], 0.0)
        with nc.allow_low_precision(reason="bf16 channel-sum for 7x7 attn"):
            for q in range(32):
                ptt = pconv.tile([128, 512], BF16, tag="c")
                for j in range(4):
                    chh = 4 * q + j
                    nc.tensor.transpose(
                        ptt[:, 128 * j:128 * (j + 1)], osb[:, chh, :], ident)
                v = ptt.rearrange("p (a b) -> p a b", a=4)
                nc.vector.tensor_reduce(
                    spmax[:, 3 + 4 * q:7 + 4 * q], v, AX.X, ALU.max)
                nc.vector.tensor_reduce(
                    spsum[:, 3 + 4 * q:7 + 4 * q], v, AX.X, ALU.add)

        psw = pconv.tile([128, 128], F32, tag="c")
        for t in range(14):
            c, dh = t // 7, t % 7
            src = spsum if c == 0 else spmax
            nc.tensor.matmul(psw, mcdh[:, t * 128:(t + 1) * 128],
                             src[:, dh:dh + 128],
                             start=(t == 0), stop=(t == 13))
        swT = spool.tile([128, 128], BF16, tag="swT")
        nc.scalar.activation(swT, psw, ACTF.Sigmoid, bias=bssa, scale=gssa)
        pswh = pconv.tile([128, 128], BF16, tag="c")
        nc.tensor.transpose(pswh, swT, ident)
        swH = spool.tile([128, 128], BF16, tag="swH")
        nc.vector.tensor_copy(swH, pswh)
        nc.sync.dma_start(ssw_d[b], swH)

    def stage_final(b):
        """out = osb*sw + x, 16-row chunks, DVE, residual from bf16 xp."""
        xp, osb = xps[b], osbs[b]
        for t in range(8):
            r0 = 16 * t
            swbc = fpool.tile([128, 16, W], BF16, tag="swbc")
            nc.sync.dma_start(
                swbc, ssw_d[b, r0:r0 + 16, :].partition_broadcast(128))
            tmul = fpool.tile([128, 16, W], BF16, tag="tmul")
            nc.vector.tensor_mul(tmul, osb[:, r0:r0 + 16, :], swbc)
            fo = fpool.tile([128, 16, W], F32, tag="fo")
            nc.vector.tensor_tensor(
                fo, tmul, xp[:, 1 + r0:1 + r0 + 16, 1:W + 1], ALU.add)
            nc.sync.dma_start(out_d[b, :, r0:r0 + 16, :], fo)

    # ---------- pipelined schedule over the two samples ----------
    rwblk0 = stage_load_pool_route(0)
    stage_wgen(0, rwblk0)
    stage_conv(0)
    rwblk1 = stage_load_pool_route(1)
    stage_wgen(1, rwblk1)
    stage_se(0)
    stage_cbam(0)
    stage_conv(1)
    stage_final(0)
    stage_se(1)
    stage_cbam(1)
    stage_final(1)


def _host_prep(inp):
    import ml_dtypes
    experts = np.ascontiguousarray(inp["experts"], dtype=np.float32)
    # ew2[og][j'*16+e][k*128+i] = experts[e, og*8+j', i, kh, kw], k=kh*3+kw
    ew = experts.reshape(E, 16, 8, CI, 9)
    ew = np.ascontiguousarray(ew.transpose(1, 2, 0, 4, 3)).reshape(16, 128, IKK)

    identb = np.eye(128, dtype=np.float32)

    # banded-Toeplitz 7x7 attention matrices:
    # mcdh[c*7+dh][k, w] = sak[c, dh, k-w+3] for |k-w+3| in [0,7), else 0
    saw = np.asarray(inp["sa_w"], np.float32).reshape(2, 7, 7)
    sak = saw.copy()
    sak[0] *= 1.0 / CO  # fold channel-mean normalization into mean taps
    mc = np.zeros((14, 128, 128), dtype=np.float32)
    kk, ww = np.meshgrid(np.arange(128), np.arange(128), indexing="ij")
    dwi = kk - ww + 3
    band = (dwi >= 0) & (dwi < 7)
    for c in range(2):
        for dh in range(7):
            m = np.zeros((128, 128), dtype=np.float32)
            m[band] = sak[c, dh, dwi[band]]
            mc[c * 7 + dh] = m
    # device layout: [128 k-partitions, 14*128]
    mcdh = np.ascontiguousarray(mc.transpose(1, 0, 2)).reshape(128, 14 * 128)

    bm = np.zeros((8, 16, 8), dtype=np.float32)
    for j in range(8):
        bm[j, :, j] = 1.0
    bm = bm.reshape(128, 8)

    shared = {
        "experts_w": ew.astype(ml_dtypes.bfloat16),
        "identb": identb.astype(ml_dtypes.bfloat16),
        "mcdh": mcdh.astype(ml_dtypes.bfloat16),
        "rw1t": np.ascontiguousarray(inp["rw1"].T, dtype=np.float32),
        "rw2t": np.ascontiguousarray(inp["rw2"].T, dtype=np.float32),
        "rw3t": np.ascontiguousarray(inp["rw3"].T, dtype=np.float32),
        "caw1t": np.ascontiguousarray(inp["ca_w1"].T, dtype=np.float32),
        "caw2t": np.ascontiguousarray(inp["ca_w2"].T, dtype=np.float32),
        "rbn1_g": np.asarray(inp["rbn1_g"], np.float32),
        "rbn1_b": np.asarray(inp["rbn1_b"], np.float32),
        "rbn2_g": np.asarray(inp["rbn2_g"], np.float32),
        "rbn2_b": np.asarray(inp["rbn2_b"], np.float32),
        "rb3": np.asarray(inp["rb3"], np.float32),
        "ca_bn1_g": np.asarray(inp["ca_bn1_g"], np.float32),
        "ca_bn1_b": np.asarray(inp["ca_bn1_b"], np.float32),
        "ca_bn2_g": np.asarray(inp["ca_bn2_g"], np.float32),
        "ca_bn2_b": np.asarray(inp["ca_bn2_b"], np.float32),
        "sa_bn_g": np.asarray(inp["sa_bn_g"], np.float32),
        "sa_bn_b": np.asarray(inp["sa_bn_b"], np.float32),
        "bmask": bm.astype(ml_dtypes.bfloat16),
    }
    x = np.asarray(inp["x"], np.float32)
    in_maps = []
    for c in range(NCORES):
        m = dict(shared)
        xc = np.ascontiguousarray(x[BL * c:BL * (c + 1)])
        m["x2b"] = xc.astype(ml_dtypes.bfloat16)
        in_maps.append(m)
    return in_maps


def get_module():
    if "nc" not in _CACHE:
        _CACHE["nc"] = _build_module()
    return _CACHE["nc"]


def kernel(**inputs):
    nc = get_module()
    in_maps = _host_prep(inputs)
    res = run_bass_kernel_spmd(nc, in_maps, core_ids=list(range(NCORES)))
    out = np.concatenate([r["out"] for r in res.results], axis=0)
    return out.astype(np.float32)
